# revision 1
# baseline (speedup 1.0000x reference)
"""Trainium2 Bass kernel for nn_Activation1d (upsample2x-linear -> SiLU -> downsample2x).

Math: with align_corners=False linear resize, UP_RATIO=2, the whole op reduces
to a 3-tap stencil along T:
    a[j] = 0.25*(3*x[j] + x[j-1])      (x[-1] clamped to x[0])
    b[j] = 0.25*(3*x[j] + x[j+1])      (x[T] clamped to x[T-1])
    out[j] = 0.5*(silu(a[j]) + silu(b[j]))

Pure pointwise over (B, C): shard B*C = 8192 rows across 8 cores, T stays local.
"""

import os
import sys
from contextlib import ExitStack

import numpy as np

for _p in ("/opt/trn_rl_repo",):
    if _p not in sys.path:
        sys.path.insert(0, _p)

import bass_rust
import concourse.bass as bass
import concourse.mybir as mybir
from concourse import tile
from concourse.bass_utils import run_bass_kernel_spmd

N_CORES = 8
B, C, T = 16, 512, 8192
ROWS = B * C                 # 8192
RPC = ROWS // N_CORES        # 1024 rows per core
P = 128                      # SBUF partitions
N_RT = RPC // P              # 8 row-tiles per core
W = 2048                     # free-dim compute chunk width
NCH = T // W                 # chunks per row-tile

ALU = mybir.AluOpType
AFT = mybir.ActivationFunctionType
F32 = mybir.dt.float32

# --- tunables (env-overridable for experiments) ---
CDT_NAME = os.environ.get("K_CDT", "float16")       # compute dtype for DVE ops
USE_STT = os.environ.get("K_STT", "1") == "1"        # scalar_tensor_tensor vs (t3 + add)
# Inputs via SWDGE (gpsimd); outputs MUST be HWDGE (sync): compute
# instructions waiting on an SWDGE out-DMA's lane semaphore hang the device
# (the +16 never lands), while the same WAR pattern on DMAHW lanes works.
OUT_DMA_ENGINE = os.environ.get("K_ODMA", "sync")
IN_DMA_ENGINE = os.environ.get("K_IDMA", "gpsimd")

_LAST_EXEC_NS = None
_LAST_RESULT = None


def _build():
    cdt = getattr(mybir.dt, CDT_NAME)
    # Tile's stale SBUF cap (192K) leaves real capacity (208K usable) unused;
    # this kernel needs ~204K per partition.
    import concourse.tile_utils as _tu

    _tu.max_sbuf_usage = 208 * 1024
    nc = bass.Bass()
    x_ext = nc.declare_dram_parameter("x", [RPC, T], F32, isOutput=False)
    o_ext = nc.declare_dram_parameter("out", [RPC, T], F32, isOutput=True)

    with tile.TileContext(nc) as tc:
        with ExitStack() as ctx:
            xpool = ctx.enter_context(tc.tile_pool(name="xp", bufs=2))
            cpool = ctx.enter_context(tc.tile_pool(name="cp", bufs=2))
            tpool = ctx.enter_context(tc.tile_pool(name="tp", bufs=3))
            opool = ctx.enter_context(tc.tile_pool(name="op", bufs=2))

            in_dma = getattr(nc, IN_DMA_ENGINE)
            out_dma = getattr(nc, OUT_DMA_ENGINE)

            # DMA budget: broken DGE lane-reuse in this stack means at most 8
            # DMAs per ring (SWDGE qPoolDynamic / HWDGE qSPDynamicHW) so no
            # lane is ever reused: 8 full-row loads (SWDGE) + 8 full-row
            # stores (sync HWDGE). The shifted copy xs runs on Pool compute.
            for r in range(N_RT):
                rows = slice(r * P, (r + 1) * P)
                xt = xpool.tile([P, T], F32, tag="xt")
                in_dma.dma_start(xt[:], x_ext[rows, :])

                # f32 -> cdt conversion on DVE (single-src 2x mode); single
                # producer keeps the Pool xs copies at one wait.
                xb = cpool.tile([P, T], cdt, tag="xb")
                nc.vector.tensor_copy(xb[:], xt[:])
                # xs[m] = xb[m-1] for m in [0, T+1], clamped at both ends.
                # xs absorbs the edge clamping and keeps every 16-bit stencil
                # operand 4-byte aligned (ACT crashes on misaligned 16-bit
                # writes; Pool handles them at line rate).
                xs = cpool.tile([P, T + 4], cdt, tag="xs")
                nc.gpsimd.tensor_copy(xs[:, 0:1], xb[:, 0:1])
                nc.gpsimd.tensor_copy(xs[:, 1 : T + 1], xb[:])
                nc.gpsimd.tensor_copy(xs[:, T + 1 : T + 2], xb[:, T - 1 : T])

                oc = opool.tile([P, T], F32, tag="oc")
                for ci in range(NCH):
                    lo, hi = ci * W, (ci + 1) * W
                    ta = tpool.tile([P, W], cdt, tag="tt")
                    tb = tpool.tile([P, W], cdt, tag="tt")
                    # ta = 3*xb + x[j-1] ; tb = 3*xb + x[j+1]
                    nc.vector.scalar_tensor_tensor(
                        ta[:], xb[:, lo:hi], 3.0, xs[:, lo : lo + W],
                        ALU.mult, ALU.add,
                    )
                    nc.vector.scalar_tensor_tensor(
                        tb[:], xb[:, lo:hi], 3.0, xs[:, lo + 2 : lo + W + 2],
                        ALU.mult, ALU.add,
                    )
                    # silu in place, with the 0.25 folded into ACT's free
                    # input scale (in-place: 1:1 elementwise, no RAW hazard)
                    nc.scalar.activation(ta[:], ta[:], AFT.Silu, scale=0.25)
                    nc.scalar.activation(tb[:], tb[:], AFT.Silu, scale=0.25)
                    # out = 0.5*(silu_a+silu_b): add in place, scale -> f32
                    nc.vector.tensor_add(ta[:], ta[:], tb[:])
                    nc.vector.tensor_scalar_mul(oc[:, lo:hi], ta[:], 0.5)
                out_dma.dma_start(o_ext[rows, :], oc[:])
    return nc


_PRUNABLE = (
    bass_rust.InstDMACopy,
    bass_rust.InstTensorCopy,
    bass_rust.InstTensorTensor,
    bass_rust.InstTensorScalarPtr,
    bass_rust.InstActivation,
)


def _transitive_prune_waits(nc):
    """Reduce every prunable instruction to at most one semaphore wait.

    This walrus build's engine/DMA ISA structs hold a single sync wait per
    instruction, but Tile's scheduler emits one wait per dependent proc
    because its vector clock is not transitively minimal across procs.

    Phase 1 simulates the emitted program (greedy topological execution over
    per-engine in-order streams), recording for every semaphore value the
    happens-before knowledge it implies and a global feasible order.
    Phase 2 drops waits implied by program order + remaining waits; if more
    than one wait survives, it strengthens one wait (raising its threshold
    to a value already reached earlier in the phase-1 order, so no cycle can
    form) until that single wait implies all the others.

    Soundness: engines complete instructions in stream order (DVE/ACT/SP);
    per-lane DMA updates land in issue order (Tile serializes lane reuse);
    Pool compute may complete out of order across Q7 cores, so no transitive
    knowledge is propagated through the Pool semaphore.
    """
    f = nc.m.functions[0]
    streams = {}
    for b in f.blocks:
        for inst in b.instructions:
            streams.setdefault(str(inst.engine), []).append(inst)

    def merge(dst, src):
        for s, v in src.items():
            if dst.get(s, 0) < v:
                dst[s] = v

    # ---- phase 1: simulate, collect logs ----
    sem_val = {}
    sem_log = {}        # sem -> list of (cum_value, knowledge, step)
    proc_know = {e: {} for e in streams}
    proc_self = {e: {} for e in streams}
    ptr = {e: 0 for e in streams}
    inst_info = {}      # id(inst) -> (base knowledge, step)
    step = 0

    def knowledge_of(sem, val, max_step=None):
        k = {sem: val}
        if sem.startswith("Pool"):
            return k
        for cum, kn, st in sem_log.get(sem, ()):
            if max_step is not None and st >= max_step:
                break
            merge(k, kn)
            if cum >= val:
                break
        return k

    def satisfied(w):
        v = sem_val.get(w.ant_name, 0)
        return v == w.wait_value if w.wait_mode == "sem-eq-imm" else v >= w.wait_value

    def execute(eng, inst):
        nonlocal step, done
        si = inst.sync_info
        waits = list(si.on_wait) if si is not None else []
        base = dict(proc_know[eng])
        merge(base, proc_self[eng])
        inst_info[id(inst)] = (dict(base), step)
        acc = base
        for w in waits:
            merge(acc, knowledge_of(w.ant_name, w.wait_value))
        proc_know[eng] = acc
        is_dma = isinstance(inst, bass_rust.InstDMACopy)
        if si is not None:
            for u in si.on_update:
                s = u.ant_name
                dv = {
                    "sem-add-imm": u.update_value,
                    "sem-inc": 1,
                    "sem-dec": -1,
                    "sem-sub-imm": -u.update_value,
                }[u.update_mode]
                nv = sem_val.get(s, 0) + dv
                sem_val[s] = nv
                kn = dict(proc_know[eng])
                merge(kn, proc_self[eng])
                if not is_dma and eng != "EngineType.Pool":
                    # Pool (8 Q7 cores) completes out of order: a later Pool
                    # instruction cannot assume earlier ones finished.
                    proc_self[eng][s] = max(proc_self[eng].get(s, 0), nv)
                kn[s] = nv
                sem_log.setdefault(s, []).append((nv, kn, step))
        ptr[eng] += 1
        done += 1
        step += 1

    total = sum(len(s) for s in streams.values())
    done, progress = 0, True
    while done < total and progress:
        progress = False
        # Execute DMAs as late as possible so compute events order before
        # them in the recorded feasible order (maximizes strengthening).
        for eng, stream in streams.items():
            while ptr[eng] < len(stream):
                inst = stream[ptr[eng]]
                si = inst.sync_info
                waits = list(si.on_wait) if si is not None else []
                if isinstance(inst, bass_rust.InstDMACopy):
                    break
                if not all(satisfied(w) for w in waits):
                    break
                execute(eng, inst)
                progress = True
        if progress:
            continue
        for eng, stream in streams.items():
            if ptr[eng] < len(stream):
                inst = stream[ptr[eng]]
                si = inst.sync_info
                waits = list(si.on_wait) if si is not None else []
                if isinstance(inst, bass_rust.InstDMACopy) and all(
                    satisfied(w) for w in waits
                ):
                    execute(eng, inst)
                    progress = True
                    break
    if done < total:
        import logging

        logging.warning(
            "_transitive_prune_waits: simulation stalled at %d/%d; "
            "no pruning applied",
            done,
            total,
        )
        return

    # ---- phase 2: prune / strengthen ----
    remaining_multi = []
    for eng, stream in streams.items():
        for inst in stream:
            si = inst.sync_info
            waits = list(si.on_wait) if si is not None else []
            if len(waits) < 2:
                continue
            if not isinstance(inst, _PRUNABLE) or any(
                w.wait_mode != "sem-ge-imm" for w in waits
            ):
                remaining_multi.append(inst)
                continue
            base, my_step = inst_info[id(inst)]

            def implied(k, ws):
                return all(k.get(w.ant_name, 0) >= w.wait_value for w in ws)

            # A DMA's wait on its own update lane (Tile's lane-reuse
            # throttle) is load-bearing for the DGE hardware beyond its
            # ordering semantics: dropping it wedges the device even when
            # the ordering is transitively guaranteed. Never touch those.
            own_lanes = set()
            if isinstance(inst, bass_rust.InstDMACopy) and si is not None:
                own_lanes = {u.ant_name for u in si.on_update}
            fixed = [w for w in waits if w.ant_name in own_lanes]
            # 1) drop waits implied by base + the other waits (greedy, all orders)
            import itertools

            best = None
            for order in itertools.permutations(range(len(waits))):
                a = dict(base)
                for w in fixed:
                    merge(a, knowledge_of(w.ant_name, w.wait_value))
                kp = [i for i in range(len(waits)) if waits[i] in fixed]
                for i in order:
                    w = waits[i]
                    if w in fixed:
                        continue
                    if a.get(w.ant_name, 0) >= w.wait_value:
                        continue
                    kp.append(i)
                    merge(a, knowledge_of(w.ant_name, w.wait_value))
                if best is None or len(kp) < len(best):
                    best = kp
                if len(kp) <= 1:
                    break
            kept = [waits[i] for i in sorted(best)]
            # 2) strengthen: find one sem whose (possibly later) value implies all
            if len(kept) > 1 and fixed:
                remaining_multi.append(inst)
                continue
            if len(kept) > 1:
                chosen = None
                cands = sorted(
                    {w.ant_name for w in waits},
                    key=lambda s: (s.startswith("DMA"), s),
                )
                for s in cands:
                    if s.startswith("Pool"):
                        continue
                    k = dict(base)
                    for cum, kn, st in sem_log.get(s, ()):
                        if st >= my_step:
                            break  # only events already ordered before us
                        merge(k, kn)
                        k[s] = max(k.get(s, 0), cum)
                        if implied(k, waits):
                            chosen = (s, cum)
                            break
                    if chosen:
                        break
                if chosen:
                    tmpl = next(w for w in waits if w.ant_name == chosen[0])
                    tmpl.wait_value = chosen[1]
                    kept = [tmpl]
                else:
                    remaining_multi.append(inst)
                    continue
            if len(kept) != len(waits) or any(
                k.wait_value != w.wait_value for k, w in zip(kept, waits)
            ):
                si.on_wait = kept
                inst.sync_info = si
    # ---- phase 3: non-prunable multi-wait instructions (the tail drain) ----
    # Reduce to the minimal wait subset via transitivity, keep one wait, and
    # move the rest onto zero-wait tail instructions (event semaphores) that
    # execute before NEFF completion. Sound: the conditions depend only on
    # DMAs issued in the main region, so no donor can deadlock, and every
    # stream must finish before the NEFF signals done.
    import itertools as _it

    unresolved = []
    if remaining_multi:
        last_dma_step = max(
            (inst_info[id(i)][1] for s in streams.values() for i in s
             if isinstance(i, bass_rust.InstDMACopy) and id(i) in inst_info),
            default=0,
        )
        donors = [
            i
            for s in streams.values()
            for i in s
            if isinstance(
                i, (bass_rust.InstEventSemaphore, bass_rust.InstDrain)
            )
            and i.sync_info is not None
            and not list(i.sync_info.on_wait)
            and inst_info.get(id(i), (None, -1))[1] > last_dma_step
        ]
        for inst in remaining_multi:
            si = inst.sync_info
            waits = list(si.on_wait)
            if any(w.wait_mode != "sem-ge-imm" for w in waits):
                unresolved.append(inst)
                continue
            base, _st = inst_info[id(inst)]
            best = None
            for r in range(1, len(waits) + 1):
                for combo in _it.combinations(range(len(waits)), r):
                    k = dict(base)
                    for i in combo:
                        merge(k, knowledge_of(waits[i].ant_name, waits[i].wait_value))
                    if all(k.get(w.ant_name, 0) >= w.wait_value for w in waits):
                        best = [waits[i] for i in combo]
                        break
                if best:
                    break
            if best is None:
                best = waits
            extra = best[1:]
            if len(extra) > len(donors):
                unresolved.append(inst)
                continue
            for w in extra:
                d = donors.pop()
                dsi = d.sync_info
                dsi.on_wait = [w]
                d.sync_info = dsi
            si.on_wait = best[:1]
            inst.sync_info = si
    if unresolved:
        import logging

        logging.warning(
            "_transitive_prune_waits: %d instructions still multi-wait: %s",
            len(unresolved),
            [i.name for i in unresolved[:10]],
        )


_NC = None


def _get_nc():
    global _NC
    if _NC is None:
        _NC = _build()
        _transitive_prune_waits(_NC)
    return _NC


def kernel(x):
    global _LAST_EXEC_NS, _LAST_RESULT
    x = np.asarray(x, dtype=np.float32)
    assert x.shape == (B, C, T), x.shape
    flat = np.ascontiguousarray(x.reshape(ROWS, T))
    in_maps = [
        {"x": np.ascontiguousarray(flat[i * RPC : (i + 1) * RPC])}
        for i in range(N_CORES)
    ]
    nc = _get_nc()
    res = run_bass_kernel_spmd(
        nc,
        in_maps,
        core_ids=list(range(N_CORES)),
        trace=os.environ.get("K_TRACE", "0") == "1",
    )
    _LAST_RESULT = res
    _LAST_EXEC_NS = res.exec_time_ns
    out = np.concatenate([r["out"] for r in res.results], axis=0)
    return np.ascontiguousarray(out.reshape(B, C, T))



# revision 8
# speedup vs baseline: 1.9835x; 1.9835x over previous
"""Trainium2 Bass kernel for nn_Activation1d (upsample2x-linear -> SiLU -> downsample2x).

Math: with align_corners=False linear resize, UP_RATIO=2, the whole op reduces
to a 3-tap stencil along T:
    a[j] = 0.25*(3*x[j] + x[j-1])      (x[-1] clamped to x[0])
    b[j] = 0.25*(3*x[j] + x[j+1])      (x[T] clamped to x[T-1])
    out[j] = 0.5*(silu(a[j]) + silu(b[j]))

Pure pointwise over (B, C): shard B*C = 8192 rows across 8 cores, T stays local.
"""

import os
import sys
from contextlib import ExitStack

import numpy as np

for _p in ("/opt/trn_rl_repo",):
    if _p not in sys.path:
        sys.path.insert(0, _p)

import bass_rust
import concourse.bass as bass
import concourse.mybir as mybir
from concourse import tile
from concourse.bass_utils import run_bass_kernel_spmd

N_CORES = 8
B, C, T = 16, 512, 8192
ROWS = B * C                 # 8192
RPC = ROWS // N_CORES        # 1024 rows per core
P = 128                      # SBUF partitions
N_RT = RPC // P              # 8 row-tiles per core
W = 2048                     # free-dim compute chunk width
NCH = T // W                 # chunks per row-tile

ALU = mybir.AluOpType
AFT = mybir.ActivationFunctionType
F32 = mybir.dt.float32

# --- tunables (env-overridable for experiments) ---
CDT_NAME = os.environ.get("K_CDT", "float16")       # compute dtype for DVE ops
USE_STT = os.environ.get("K_STT", "1") == "1"        # scalar_tensor_tensor vs (t3 + add)
# Inputs via SWDGE (gpsimd); outputs MUST be HWDGE (sync): compute
# instructions waiting on an SWDGE out-DMA's lane semaphore hang the device
# (the +16 never lands), while the same WAR pattern on DMAHW lanes works.
OUT_DMA_ENGINE = os.environ.get("K_ODMA", "sync")
IN_DMA_ENGINE = os.environ.get("K_IDMA", "gpsimd")

_LAST_EXEC_NS = None
_LAST_RESULT = None


def _build():
    cdt = getattr(mybir.dt, CDT_NAME)
    # Tile's stale SBUF cap (192K) leaves real capacity (208K usable) unused;
    # this kernel needs ~197K per partition.
    import concourse.tile_utils as _tu

    _tu.max_sbuf_usage = 208 * 1024
    nc = bass.Bass()
    x_ext = nc.declare_dram_parameter("x", [RPC, T], F32, isOutput=False)
    o_ext = nc.declare_dram_parameter("out", [RPC, T], F32, isOutput=True)

    with tile.TileContext(nc) as tc:
        with ExitStack() as ctx:
            xpool = ctx.enter_context(tc.tile_pool(name="xp", bufs=2))
            tpool = ctx.enter_context(tc.tile_pool(name="tp", bufs=2))
            opool = ctx.enter_context(tc.tile_pool(name="op", bufs=2))

            in_dma = getattr(nc, IN_DMA_ENGINE)
            out_dma = getattr(nc, OUT_DMA_ENGINE)

            # [P,1] scalar-slot constants for the custom-DVE ops (the TTSS
            # scale slots want DATA_PTR APs; float immediates mis-encode).
            cpool = ctx.enter_context(tc.tile_pool(name="cp", bufs=1))
            c3 = cpool.tile([P, 1], F32, tag="c3")
            c0 = cpool.tile([P, 1], F32, tag="c0")
            cm1 = cpool.tile([P, 1], F32, tag="cm1")
            nc.vector.memset(c3[:], 3.0)
            nc.vector.memset(c0[:], 0.0)
            nc.vector.memset(cm1[:], -1.0)

            # DMA budget: broken DGE lane-reuse in this stack means at most 8
            # DMAs per ring (SWDGE qPoolDynamic / HWDGE qSPDynamicHW) so no
            # lane is ever reused: 8 full-row loads (SWDGE) + 8 full-row
            # stores (sync HWDGE).
            for r in range(N_RT):
                rows = slice(r * P, (r + 1) * P)
                # xt[m] = x[m-1], one element of clamp padding on each side:
                # every +-1-shifted stencil read below is then an aligned f32
                # view into xt -- no shifted SBUF copy needed at all.
                xt = xpool.tile([P, T + 2], F32, tag="xt")
                in_dma.dma_start(xt[:, 1 : T + 1], x_ext[rows, :])
                # Edge clamp fills on DVE so the ATAs below depend on them
                # via program order: each InstISA custom-DVE op can encode
                # only ONE semaphore wait, so ATA must not need both a DMA
                # lane wait and a Pool wait.
                nc.vector.tensor_copy(xt[:, 0:1], xt[:, 1:2])
                nc.vector.tensor_copy(xt[:, T + 1 : T + 2], xt[:, T : T + 1])

                # ta = 3*x[j] + x[j-1], tb = 3*x[j] + x[j+1]; one custom-DVE
                # op each (f32 in, f16 out), replacing the STT pair that ran
                # at ~3.6 cyc/el.
                ta = tpool.tile([P, T], cdt, tag="ta")
                tb = tpool.tile([P, T], cdt, tag="tb")
                nc.vector.affine_then_add(
                    ta[:], xt[:, 1 : T + 1], xt[:, 0:T], c3[:], c0[:]
                )
                nc.vector.affine_then_add(
                    tb[:], xt[:, 1 : T + 1], xt[:, 2 : T + 2], c3[:], c0[:]
                )
                # silu in place, with the 0.25 folded into ACT's free input
                # scale (in-place: 1:1 elementwise, no RAW hazard)
                nc.scalar.activation(ta[:], ta[:], AFT.Silu, scale=0.25)
                nc.scalar.activation(tb[:], tb[:], AFT.Silu, scale=0.25)
                # out = (ta - tb*(-1) - 0)*0.5 = 0.5*(silu_a + silu_b),
                # f16 in -> f32 out in a single DVE op
                oc = opool.tile([P, T], F32, tag="oc")
                # Dummy first-writer claim of oc: absorbs the WAR wait on the
                # out-DMA lane (r-2) here, so ln_bwd_dx itself carries only
                # the ACT RAW wait (InstISA holds a single wait).
                nc.vector.memset(oc[:, 0:1], 0.0)
                nc.vector.ln_bwd_dx(oc[:], ta[:], tb[:], cm1[:], c0[:], scale=0.5)
                out_dma.dma_start(o_ext[rows, :], oc[:])
    return nc


_PRUNABLE = (
    bass_rust.InstDMACopy,
    bass_rust.InstTensorCopy,
    bass_rust.InstTensorTensor,
    bass_rust.InstTensorScalarPtr,
    bass_rust.InstActivation,
    bass_rust.InstCustomDveAnt,
)


def _transitive_prune_waits(nc):
    """Reduce every prunable instruction to at most one semaphore wait.

    This walrus build's engine/DMA ISA structs hold a single sync wait per
    instruction, but Tile's scheduler emits one wait per dependent proc
    because its vector clock is not transitively minimal across procs.

    Phase 1 simulates the emitted program (greedy topological execution over
    per-engine in-order streams), recording for every semaphore value the
    happens-before knowledge it implies and a global feasible order.
    Phase 2 drops waits implied by program order + remaining waits; if more
    than one wait survives, it strengthens one wait (raising its threshold
    to a value already reached earlier in the phase-1 order, so no cycle can
    form) until that single wait implies all the others.

    Soundness: engines complete instructions in stream order (DVE/ACT/SP);
    per-lane DMA updates land in issue order (Tile serializes lane reuse);
    Pool compute may complete out of order across Q7 cores, so no transitive
    knowledge is propagated through the Pool semaphore.
    """
    f = nc.m.functions[0]
    streams = {}
    for b in f.blocks:
        for inst in b.instructions:
            streams.setdefault(str(inst.engine), []).append(inst)

    def merge(dst, src):
        for s, v in src.items():
            if dst.get(s, 0) < v:
                dst[s] = v

    # ---- phase 1: simulate, collect logs ----
    sem_val = {}
    sem_log = {}        # sem -> list of (cum_value, knowledge, step)
    proc_know = {e: {} for e in streams}
    proc_self = {e: {} for e in streams}
    ptr = {e: 0 for e in streams}
    inst_info = {}      # id(inst) -> (base knowledge, step)
    step = 0

    def knowledge_of(sem, val, max_step=None):
        k = {sem: val}
        if sem.startswith("Pool"):
            return k
        for cum, kn, st in sem_log.get(sem, ()):
            if max_step is not None and st >= max_step:
                break
            merge(k, kn)
            if cum >= val:
                break
        return k

    def satisfied(w):
        v = sem_val.get(w.ant_name, 0)
        return v == w.wait_value if w.wait_mode == "sem-eq-imm" else v >= w.wait_value

    def execute(eng, inst):
        nonlocal step, done
        si = inst.sync_info
        waits = list(si.on_wait) if si is not None else []
        base = dict(proc_know[eng])
        merge(base, proc_self[eng])
        inst_info[id(inst)] = (dict(base), step)
        acc = base
        for w in waits:
            merge(acc, knowledge_of(w.ant_name, w.wait_value))
        proc_know[eng] = acc
        is_dma = isinstance(inst, bass_rust.InstDMACopy)
        if si is not None:
            for u in si.on_update:
                s = u.ant_name
                dv = {
                    "sem-add-imm": u.update_value,
                    "sem-inc": 1,
                    "sem-dec": -1,
                    "sem-sub-imm": -u.update_value,
                }[u.update_mode]
                nv = sem_val.get(s, 0) + dv
                sem_val[s] = nv
                kn = dict(proc_know[eng])
                merge(kn, proc_self[eng])
                if not is_dma and eng != "EngineType.Pool":
                    # Pool (8 Q7 cores) completes out of order: a later Pool
                    # instruction cannot assume earlier ones finished.
                    proc_self[eng][s] = max(proc_self[eng].get(s, 0), nv)
                kn[s] = nv
                sem_log.setdefault(s, []).append((nv, kn, step))
        ptr[eng] += 1
        done += 1
        step += 1

    total = sum(len(s) for s in streams.values())
    done, progress = 0, True
    while done < total and progress:
        progress = False
        # Execute DMAs as late as possible so compute events order before
        # them in the recorded feasible order (maximizes strengthening).
        for eng, stream in streams.items():
            while ptr[eng] < len(stream):
                inst = stream[ptr[eng]]
                si = inst.sync_info
                waits = list(si.on_wait) if si is not None else []
                if isinstance(inst, bass_rust.InstDMACopy):
                    break
                if not all(satisfied(w) for w in waits):
                    break
                execute(eng, inst)
                progress = True
        if progress:
            continue
        for eng, stream in streams.items():
            if ptr[eng] < len(stream):
                inst = stream[ptr[eng]]
                si = inst.sync_info
                waits = list(si.on_wait) if si is not None else []
                if isinstance(inst, bass_rust.InstDMACopy) and all(
                    satisfied(w) for w in waits
                ):
                    execute(eng, inst)
                    progress = True
                    break
    if done < total:
        import logging

        logging.warning(
            "_transitive_prune_waits: simulation stalled at %d/%d; "
            "no pruning applied",
            done,
            total,
        )
        return

    # ---- phase 2: prune / strengthen ----
    remaining_multi = []
    for eng, stream in streams.items():
        for inst in stream:
            si = inst.sync_info
            waits = list(si.on_wait) if si is not None else []
            if len(waits) < 2:
                continue
            if not isinstance(inst, _PRUNABLE) or any(
                w.wait_mode != "sem-ge-imm" for w in waits
            ):
                remaining_multi.append(inst)
                continue
            base, my_step = inst_info[id(inst)]

            def implied(k, ws):
                return all(k.get(w.ant_name, 0) >= w.wait_value for w in ws)

            # A DMA's wait on its own update lane (Tile's lane-reuse
            # throttle) is load-bearing for the DGE hardware beyond its
            # ordering semantics: dropping it wedges the device even when
            # the ordering is transitively guaranteed. Never touch those.
            own_lanes = set()
            if isinstance(inst, bass_rust.InstDMACopy) and si is not None:
                own_lanes = {u.ant_name for u in si.on_update}
            fixed = [w for w in waits if w.ant_name in own_lanes]
            # 1) drop waits implied by base + the other waits (greedy, all orders)
            import itertools

            best = None
            for order in itertools.permutations(range(len(waits))):
                a = dict(base)
                for w in fixed:
                    merge(a, knowledge_of(w.ant_name, w.wait_value))
                kp = [i for i in range(len(waits)) if waits[i] in fixed]
                for i in order:
                    w = waits[i]
                    if w in fixed:
                        continue
                    if a.get(w.ant_name, 0) >= w.wait_value:
                        continue
                    kp.append(i)
                    merge(a, knowledge_of(w.ant_name, w.wait_value))
                if best is None or len(kp) < len(best):
                    best = kp
                if len(kp) <= 1:
                    break
            kept = [waits[i] for i in sorted(best)]
            # 2) strengthen: find one sem whose (possibly later) value implies all
            if len(kept) > 1 and fixed:
                remaining_multi.append(inst)
                continue
            if len(kept) > 1:
                chosen = None
                cands = sorted(
                    {w.ant_name for w in waits},
                    key=lambda s: (s.startswith("DMA"), s),
                )
                for s in cands:
                    if s.startswith("Pool"):
                        continue
                    k = dict(base)
                    for cum, kn, st in sem_log.get(s, ()):
                        if st >= my_step:
                            break  # only events already ordered before us
                        merge(k, kn)
                        k[s] = max(k.get(s, 0), cum)
                        if implied(k, waits):
                            chosen = (s, cum)
                            break
                    if chosen:
                        break
                if chosen:
                    tmpl = next(w for w in waits if w.ant_name == chosen[0])
                    tmpl.wait_value = chosen[1]
                    kept = [tmpl]
                else:
                    remaining_multi.append(inst)
                    continue
            if len(kept) != len(waits) or any(
                k.wait_value != w.wait_value for k, w in zip(kept, waits)
            ):
                si.on_wait = kept
                inst.sync_info = si
    # ---- phase 3: non-prunable multi-wait instructions (the tail drain) ----
    # Reduce to the minimal wait subset via transitivity, keep one wait, and
    # move the rest onto zero-wait tail instructions (event semaphores) that
    # execute before NEFF completion. Sound: the conditions depend only on
    # DMAs issued in the main region, so no donor can deadlock, and every
    # stream must finish before the NEFF signals done.
    import itertools as _it

    unresolved = []
    if remaining_multi:
        last_dma_step = max(
            (inst_info[id(i)][1] for s in streams.values() for i in s
             if isinstance(i, bass_rust.InstDMACopy) and id(i) in inst_info),
            default=0,
        )
        donors = [
            i
            for s in streams.values()
            for i in s
            if isinstance(
                i, (bass_rust.InstEventSemaphore, bass_rust.InstDrain)
            )
            and i.sync_info is not None
            and not list(i.sync_info.on_wait)
            and inst_info.get(id(i), (None, -1))[1] > last_dma_step
        ]
        for inst in remaining_multi:
            si = inst.sync_info
            waits = list(si.on_wait)
            if any(w.wait_mode != "sem-ge-imm" for w in waits):
                unresolved.append(inst)
                continue
            base, _st = inst_info[id(inst)]
            best = None
            for r in range(1, len(waits) + 1):
                for combo in _it.combinations(range(len(waits)), r):
                    k = dict(base)
                    for i in combo:
                        merge(k, knowledge_of(waits[i].ant_name, waits[i].wait_value))
                    if all(k.get(w.ant_name, 0) >= w.wait_value for w in waits):
                        best = [waits[i] for i in combo]
                        break
                if best:
                    break
            if best is None:
                best = waits
            extra = best[1:]
            if len(extra) > len(donors):
                unresolved.append(inst)
                continue
            for w in extra:
                d = donors.pop()
                dsi = d.sync_info
                dsi.on_wait = [w]
                d.sync_info = dsi
            si.on_wait = best[:1]
            inst.sync_info = si
    if unresolved:
        import logging

        logging.warning(
            "_transitive_prune_waits: %d instructions still multi-wait: %s",
            len(unresolved),
            [i.name for i in unresolved[:10]],
        )


_NC = None


def _get_nc():
    global _NC
    if _NC is None:
        _NC = _build()
        _transitive_prune_waits(_NC)
        # Populate .instr bytes for InstISA subclasses (custom-DVE ops).
        # Raw Bass doesn't run this pass; without it the NEFF compiler sees
        # empty .instr -> "ISA wrong length". Must run AFTER wait pruning:
        # the encoder bakes sync_info in and asserts <=1 wait per InstISA.
        from concourse.library_overlay import lower_extended_insts

        lower_extended_insts(_NC)
    return _NC


def kernel(x):
    global _LAST_EXEC_NS, _LAST_RESULT
    x = np.asarray(x, dtype=np.float32)
    assert x.shape == (B, C, T), x.shape
    flat = np.ascontiguousarray(x.reshape(ROWS, T))
    in_maps = [
        {"x": np.ascontiguousarray(flat[i * RPC : (i + 1) * RPC])}
        for i in range(N_CORES)
    ]
    nc = _get_nc()
    res = run_bass_kernel_spmd(
        nc,
        in_maps,
        core_ids=list(range(N_CORES)),
        trace=os.environ.get("K_TRACE", "0") == "1",
    )
    _LAST_RESULT = res
    _LAST_EXEC_NS = res.exec_time_ns
    out = np.concatenate([r["out"] for r in res.results], axis=0)
    return np.ascontiguousarray(out.reshape(B, C, T))



# revision 10
# speedup vs baseline: 2.0594x; 1.0383x over previous
"""Trainium2 Bass kernel for nn_Activation1d (upsample2x-linear -> SiLU -> downsample2x).

Math: with align_corners=False linear resize, UP_RATIO=2, the whole op reduces
to a 3-tap stencil along T:
    a[j] = 0.25*(3*x[j] + x[j-1])      (x[-1] clamped to x[0])
    b[j] = 0.25*(3*x[j] + x[j+1])      (x[T] clamped to x[T-1])
    out[j] = 0.5*(silu(a[j]) + silu(b[j]))

Pure pointwise over (B, C): shard B*C = 8192 rows across 8 cores, T stays local.
"""

import os
import sys
from contextlib import ExitStack

import numpy as np

for _p in ("/opt/trn_rl_repo",):
    if _p not in sys.path:
        sys.path.insert(0, _p)

import bass_rust
import concourse.bass as bass
import concourse.mybir as mybir
from concourse import tile
from concourse.bass_utils import run_bass_kernel_spmd

N_CORES = 8
B, C, T = 16, 512, 8192
ROWS = B * C                 # 8192
RPC = ROWS // N_CORES        # 1024 rows per core
P = 128                      # SBUF partitions
N_RT = RPC // P              # 8 row-tiles per core
W = 2048                     # free-dim compute chunk width
NCH = T // W                 # chunks per row-tile

ALU = mybir.AluOpType
AFT = mybir.ActivationFunctionType
F32 = mybir.dt.float32

# --- tunables (env-overridable for experiments) ---
CDT_NAME = os.environ.get("K_CDT", "float16")       # compute dtype for DVE ops
USE_STT = os.environ.get("K_STT", "1") == "1"        # scalar_tensor_tensor vs (t3 + add)
# Inputs via SWDGE (gpsimd); outputs MUST be HWDGE (sync): compute
# instructions waiting on an SWDGE out-DMA's lane semaphore hang the device
# (the +16 never lands), while the same WAR pattern on DMAHW lanes works.
OUT_DMA_ENGINE = os.environ.get("K_ODMA", "sync")
IN_DMA_ENGINE = os.environ.get("K_IDMA", "gpsimd")

_LAST_EXEC_NS = None
_LAST_RESULT = None


def _build():
    cdt = getattr(mybir.dt, CDT_NAME)
    # Tile's stale SBUF cap (192K) leaves real capacity (208K usable) unused;
    # this kernel needs ~197K per partition.
    import concourse.tile_utils as _tu

    _tu.max_sbuf_usage = 208 * 1024
    nc = bass.Bass()
    x_ext = nc.declare_dram_parameter("x", [RPC, T], F32, isOutput=False)
    o_ext = nc.declare_dram_parameter("out", [RPC, T], F32, isOutput=True)

    with tile.TileContext(nc) as tc:
        with ExitStack() as ctx:
            xpool = ctx.enter_context(tc.tile_pool(name="xp", bufs=2))
            tpool = ctx.enter_context(tc.tile_pool(name="tp", bufs=2))
            opool = ctx.enter_context(tc.tile_pool(name="op", bufs=2))

            in_dma = getattr(nc, IN_DMA_ENGINE)
            out_dma = getattr(nc, OUT_DMA_ENGINE)

            # [P,1] scalar-slot constants for the custom-DVE ops (the TTSS
            # scale slots want DATA_PTR APs; float immediates mis-encode).
            cpool = ctx.enter_context(tc.tile_pool(name="cp", bufs=1))
            c3 = cpool.tile([P, 1], F32, tag="c3")
            c0 = cpool.tile([P, 1], F32, tag="c0")
            nc.vector.memset(c3[:], 3.0)
            nc.vector.memset(c0[:], 0.0)

            # DMA budget: broken DGE lane-reuse in this stack means at most 8
            # DMAs per ring (SWDGE qPoolDynamic / HWDGE qSPDynamicHW) so no
            # lane is ever reused: 8 full-row loads (SWDGE) + 8 full-row
            # stores (sync HWDGE).
            for r in range(N_RT):
                rows = slice(r * P, (r + 1) * P)
                # xt[m] = x[m-1], one element of clamp padding on each side:
                # every +-1-shifted stencil read below is then an aligned f32
                # view into xt -- no shifted SBUF copy needed at all.
                xt = xpool.tile([P, T + 2], F32, tag="xt")
                in_dma.dma_start(xt[:, 1 : T + 1], x_ext[rows, :])
                # Edge clamp fills on DVE so the ATAs below depend on them
                # via program order: each InstISA custom-DVE op can encode
                # only ONE semaphore wait, so ATA must not need both a DMA
                # lane wait and a Pool wait.
                nc.vector.tensor_copy(xt[:, 0:1], xt[:, 1:2])
                nc.vector.tensor_copy(xt[:, T + 1 : T + 2], xt[:, T : T + 1])

                # ta = 3*x[j] + x[j-1], tb = 3*x[j] + x[j+1]; one custom-DVE
                # op each (f32 in, f16 out), replacing the STT pair that ran
                # at ~3.6 cyc/el.
                ta = tpool.tile([P, T], cdt, tag="ta")
                tb = tpool.tile([P, T], cdt, tag="tb")
                nc.vector.affine_then_add(
                    ta[:], xt[:, 1 : T + 1], xt[:, 0:T], c3[:], c0[:]
                )
                nc.vector.affine_then_add(
                    tb[:], xt[:, 1 : T + 1], xt[:, 2 : T + 2], c3[:], c0[:]
                )
                # Dummy first-writer claim of oc on DVE: absorbs the WAR wait
                # on the out-DMA lane (r-2) here, so the ACT finisher below
                # carries only one DVE-sem wait (walrus encodes <=1 wait).
                oc = opool.tile([P, T], F32, tag="oc")
                nc.vector.memset(oc[:, 0:1], 0.0)
                # silu in place, with the 0.25 folded into ACT's free input
                # scale (in-place: 1:1 elementwise, no RAW hazard)
                nc.scalar.activation(ta[:], ta[:], AFT.Silu, scale=0.25)
                nc.scalar.activation(tb[:], tb[:], AFT.Silu, scale=0.25)
                # sum on DVE at the fast f16 2x rate, then the *0.5 with the
                # f32 upconvert on ACT (Copy activation with scale) to keep
                # DVE and ACT balanced at ~21 us/tile each.
                nc.vector.tensor_add(ta[:], ta[:], tb[:])
                nc.scalar.mul(oc[:], ta[:], 0.5)
                out_dma.dma_start(o_ext[rows, :], oc[:])
    return nc


_PRUNABLE = (
    bass_rust.InstDMACopy,
    bass_rust.InstTensorCopy,
    bass_rust.InstTensorTensor,
    bass_rust.InstTensorScalarPtr,
    bass_rust.InstActivation,
    bass_rust.InstCustomDveAnt,
)


def _transitive_prune_waits(nc):
    """Reduce every prunable instruction to at most one semaphore wait.

    This walrus build's engine/DMA ISA structs hold a single sync wait per
    instruction, but Tile's scheduler emits one wait per dependent proc
    because its vector clock is not transitively minimal across procs.

    Phase 1 simulates the emitted program (greedy topological execution over
    per-engine in-order streams), recording for every semaphore value the
    happens-before knowledge it implies and a global feasible order.
    Phase 2 drops waits implied by program order + remaining waits; if more
    than one wait survives, it strengthens one wait (raising its threshold
    to a value already reached earlier in the phase-1 order, so no cycle can
    form) until that single wait implies all the others.

    Soundness: engines complete instructions in stream order (DVE/ACT/SP);
    per-lane DMA updates land in issue order (Tile serializes lane reuse);
    Pool compute may complete out of order across Q7 cores, so no transitive
    knowledge is propagated through the Pool semaphore.
    """
    f = nc.m.functions[0]
    streams = {}
    for b in f.blocks:
        for inst in b.instructions:
            streams.setdefault(str(inst.engine), []).append(inst)

    def merge(dst, src):
        for s, v in src.items():
            if dst.get(s, 0) < v:
                dst[s] = v

    # ---- phase 1: simulate, collect logs ----
    sem_val = {}
    sem_log = {}        # sem -> list of (cum_value, knowledge, step)
    proc_know = {e: {} for e in streams}
    proc_self = {e: {} for e in streams}
    ptr = {e: 0 for e in streams}
    inst_info = {}      # id(inst) -> (base knowledge, step)
    step = 0

    def knowledge_of(sem, val, max_step=None):
        k = {sem: val}
        if sem.startswith("Pool"):
            return k
        for cum, kn, st in sem_log.get(sem, ()):
            if max_step is not None and st >= max_step:
                break
            merge(k, kn)
            if cum >= val:
                break
        return k

    def satisfied(w):
        v = sem_val.get(w.ant_name, 0)
        return v == w.wait_value if w.wait_mode == "sem-eq-imm" else v >= w.wait_value

    def execute(eng, inst):
        nonlocal step, done
        si = inst.sync_info
        waits = list(si.on_wait) if si is not None else []
        base = dict(proc_know[eng])
        merge(base, proc_self[eng])
        inst_info[id(inst)] = (dict(base), step)
        acc = base
        for w in waits:
            merge(acc, knowledge_of(w.ant_name, w.wait_value))
        proc_know[eng] = acc
        is_dma = isinstance(inst, bass_rust.InstDMACopy)
        if si is not None:
            for u in si.on_update:
                s = u.ant_name
                dv = {
                    "sem-add-imm": u.update_value,
                    "sem-inc": 1,
                    "sem-dec": -1,
                    "sem-sub-imm": -u.update_value,
                }[u.update_mode]
                nv = sem_val.get(s, 0) + dv
                sem_val[s] = nv
                kn = dict(proc_know[eng])
                merge(kn, proc_self[eng])
                if not is_dma and eng != "EngineType.Pool":
                    # Pool (8 Q7 cores) completes out of order: a later Pool
                    # instruction cannot assume earlier ones finished.
                    proc_self[eng][s] = max(proc_self[eng].get(s, 0), nv)
                kn[s] = nv
                sem_log.setdefault(s, []).append((nv, kn, step))
        ptr[eng] += 1
        done += 1
        step += 1

    total = sum(len(s) for s in streams.values())
    done, progress = 0, True
    while done < total and progress:
        progress = False
        # Execute DMAs as late as possible so compute events order before
        # them in the recorded feasible order (maximizes strengthening).
        for eng, stream in streams.items():
            while ptr[eng] < len(stream):
                inst = stream[ptr[eng]]
                si = inst.sync_info
                waits = list(si.on_wait) if si is not None else []
                if isinstance(inst, bass_rust.InstDMACopy):
                    break
                if not all(satisfied(w) for w in waits):
                    break
                execute(eng, inst)
                progress = True
        if progress:
            continue
        for eng, stream in streams.items():
            if ptr[eng] < len(stream):
                inst = stream[ptr[eng]]
                si = inst.sync_info
                waits = list(si.on_wait) if si is not None else []
                if isinstance(inst, bass_rust.InstDMACopy) and all(
                    satisfied(w) for w in waits
                ):
                    execute(eng, inst)
                    progress = True
                    break
    if done < total:
        import logging

        logging.warning(
            "_transitive_prune_waits: simulation stalled at %d/%d; "
            "no pruning applied",
            done,
            total,
        )
        return

    # ---- phase 2: prune / strengthen ----
    remaining_multi = []
    for eng, stream in streams.items():
        for inst in stream:
            si = inst.sync_info
            waits = list(si.on_wait) if si is not None else []
            if len(waits) < 2:
                continue
            if not isinstance(inst, _PRUNABLE) or any(
                w.wait_mode != "sem-ge-imm" for w in waits
            ):
                remaining_multi.append(inst)
                continue
            base, my_step = inst_info[id(inst)]

            def implied(k, ws):
                return all(k.get(w.ant_name, 0) >= w.wait_value for w in ws)

            # A DMA's wait on its own update lane (Tile's lane-reuse
            # throttle) is load-bearing for the DGE hardware beyond its
            # ordering semantics: dropping it wedges the device even when
            # the ordering is transitively guaranteed. Never touch those.
            own_lanes = set()
            if isinstance(inst, bass_rust.InstDMACopy) and si is not None:
                own_lanes = {u.ant_name for u in si.on_update}
            fixed = [w for w in waits if w.ant_name in own_lanes]
            # 1) drop waits implied by base + the other waits (greedy, all orders)
            import itertools

            best = None
            for order in itertools.permutations(range(len(waits))):
                a = dict(base)
                for w in fixed:
                    merge(a, knowledge_of(w.ant_name, w.wait_value))
                kp = [i for i in range(len(waits)) if waits[i] in fixed]
                for i in order:
                    w = waits[i]
                    if w in fixed:
                        continue
                    if a.get(w.ant_name, 0) >= w.wait_value:
                        continue
                    kp.append(i)
                    merge(a, knowledge_of(w.ant_name, w.wait_value))
                if best is None or len(kp) < len(best):
                    best = kp
                if len(kp) <= 1:
                    break
            kept = [waits[i] for i in sorted(best)]
            # 2) strengthen: find one sem whose (possibly later) value implies all
            if len(kept) > 1 and fixed:
                remaining_multi.append(inst)
                continue
            if len(kept) > 1:
                chosen = None
                cands = sorted(
                    {w.ant_name for w in waits},
                    key=lambda s: (s.startswith("DMA"), s),
                )
                for s in cands:
                    if s.startswith("Pool"):
                        continue
                    k = dict(base)
                    for cum, kn, st in sem_log.get(s, ()):
                        if st >= my_step:
                            break  # only events already ordered before us
                        merge(k, kn)
                        k[s] = max(k.get(s, 0), cum)
                        if implied(k, waits):
                            chosen = (s, cum)
                            break
                    if chosen:
                        break
                if chosen:
                    tmpl = next(w for w in waits if w.ant_name == chosen[0])
                    tmpl.wait_value = chosen[1]
                    kept = [tmpl]
                else:
                    remaining_multi.append(inst)
                    continue
            if len(kept) != len(waits) or any(
                k.wait_value != w.wait_value for k, w in zip(kept, waits)
            ):
                si.on_wait = kept
                inst.sync_info = si
    # ---- phase 3: non-prunable multi-wait instructions (the tail drain) ----
    # Reduce to the minimal wait subset via transitivity, keep one wait, and
    # move the rest onto zero-wait tail instructions (event semaphores) that
    # execute before NEFF completion. Sound: the conditions depend only on
    # DMAs issued in the main region, so no donor can deadlock, and every
    # stream must finish before the NEFF signals done.
    import itertools as _it

    unresolved = []
    if remaining_multi:
        last_dma_step = max(
            (inst_info[id(i)][1] for s in streams.values() for i in s
             if isinstance(i, bass_rust.InstDMACopy) and id(i) in inst_info),
            default=0,
        )
        donors = [
            i
            for s in streams.values()
            for i in s
            if isinstance(
                i, (bass_rust.InstEventSemaphore, bass_rust.InstDrain)
            )
            and i.sync_info is not None
            and not list(i.sync_info.on_wait)
            and inst_info.get(id(i), (None, -1))[1] > last_dma_step
        ]
        for inst in remaining_multi:
            si = inst.sync_info
            waits = list(si.on_wait)
            if any(w.wait_mode != "sem-ge-imm" for w in waits):
                unresolved.append(inst)
                continue
            base, _st = inst_info[id(inst)]
            best = None
            for r in range(1, len(waits) + 1):
                for combo in _it.combinations(range(len(waits)), r):
                    k = dict(base)
                    for i in combo:
                        merge(k, knowledge_of(waits[i].ant_name, waits[i].wait_value))
                    if all(k.get(w.ant_name, 0) >= w.wait_value for w in waits):
                        best = [waits[i] for i in combo]
                        break
                if best:
                    break
            if best is None:
                best = waits
            extra = best[1:]
            if len(extra) > len(donors):
                unresolved.append(inst)
                continue
            for w in extra:
                d = donors.pop()
                dsi = d.sync_info
                dsi.on_wait = [w]
                d.sync_info = dsi
            si.on_wait = best[:1]
            inst.sync_info = si
    if unresolved:
        import logging

        logging.warning(
            "_transitive_prune_waits: %d instructions still multi-wait: %s",
            len(unresolved),
            [i.name for i in unresolved[:10]],
        )


_NC = None


def _get_nc():
    global _NC
    if _NC is None:
        _NC = _build()
        _transitive_prune_waits(_NC)
        # Populate .instr bytes for InstISA subclasses (custom-DVE ops).
        # Raw Bass doesn't run this pass; without it the NEFF compiler sees
        # empty .instr -> "ISA wrong length". Must run AFTER wait pruning:
        # the encoder bakes sync_info in and asserts <=1 wait per InstISA.
        from concourse.library_overlay import lower_extended_insts

        lower_extended_insts(_NC)
    return _NC


def kernel(x):
    global _LAST_EXEC_NS, _LAST_RESULT
    x = np.asarray(x, dtype=np.float32)
    assert x.shape == (B, C, T), x.shape
    flat = np.ascontiguousarray(x.reshape(ROWS, T))
    in_maps = [
        {"x": np.ascontiguousarray(flat[i * RPC : (i + 1) * RPC])}
        for i in range(N_CORES)
    ]
    nc = _get_nc()
    res = run_bass_kernel_spmd(
        nc,
        in_maps,
        core_ids=list(range(N_CORES)),
        trace=os.environ.get("K_TRACE", "0") == "1",
    )
    _LAST_RESULT = res
    _LAST_EXEC_NS = res.exec_time_ns
    out = np.concatenate([r["out"] for r in res.results], axis=0)
    return np.ascontiguousarray(out.reshape(B, C, T))

